# revision 1
# baseline (speedup 1.0000x reference)
"""Trainium2 Bass kernel for CompositionModel (gnn_message_passing).

Model: per-cell MLP over [log1p(X) ++ Z[cell_to_batch]] followed by a
segment-mean over batch labels.

Strategy:
  * Host: sort cells by segment id, pad each segment run to a multiple of 64
    so every 64-cell "minichunk" is single-segment; gather Z rows per cell;
    ship everything transposed (features on partitions) in bf16, blocked as
    [P, 512]-column blocks; two blocks share one DMA/log1p pass.
  * Device (8 cores, data-parallel over cells, identical static program):
      log1p (ACT Ln, 1024 cols/op) -> L1 matmul (K=128 X-part + K=32 Z-part,
      bf16) -> bias+ReLU -> fp8 h1 -> L2 as fp8 DoubleRow matmuls against
      W2 split into a (hi, lo) fp8 pair sharing one x64 scale (W2 is then
      effectively exact; only h1 carries fp8 rounding, which averages out
      in the segment mean) -> fused bias+ReLU+cast on DVE -> GpSimd
      pre-folds each minichunk in half -> grouped DVE tensor_reduce.
      The third (linear) layer commutes with the segment sum and is applied
    on the host to the 512x256 segment sums instead of 500k cells.
  * Host epilogue: subtract the (identical, analytically known) contribution
    of pad cells, scatter-add minichunk sums into segment sums, undo the x64
    W2 scale, apply W3/b3 and divide by true counts.
"""

import numpy as np
import ml_dtypes

import concourse.bacc as bacc
import concourse.mybir as mybir
import concourse.tile as tile
from concourse.bass_utils import run_bass_kernel_spmd

BF16 = ml_dtypes.bfloat16
FP8 = ml_dtypes.float8_e4m3fn

N_CORES = 8
DX = 128
DZ = 32
H = 256
B = 512
MC = 64            # minichunk: cells per single-segment group
BLK = 512          # cells per device block (matmul moving free dim)
NBLK = 126         # blocks per core (fits the fixed reference input)
W2SCALE = 64.0     # fp8 pre-scale on W2/b2, divided out on the host

_compiled = {}
_last_in_maps = None


def _build_program(nblk):
    f32 = mybir.dt.float32
    bf16 = mybir.dt.bfloat16
    fp8 = mybir.dt.float8e4
    Alu = mybir.AluOpType
    Act = mybir.ActivationFunctionType
    DR = mybir.MatmulPerfMode.DoubleRow
    mc_per_core = nblk * (BLK // MC)

    nc = bacc.Bacc("TRN2", target_bir_lowering=False, debug=False,
                   num_devices=N_CORES)

    xt_d = nc.dram_tensor("xt", [nblk // 2, DX, 2 * BLK], bf16,
                          kind="ExternalInput")
    zct_d = nc.dram_tensor("zct", [nblk, DZ, BLK], bf16, kind="ExternalInput")
    w1x_d = nc.dram_tensor("w1x", [DX, H], bf16, kind="ExternalInput")
    w1z_d = nc.dram_tensor("w1z", [DZ, H], bf16, kind="ExternalInput")
    # [m-half][hi/lo][p, ktile*128] fp8, pre-scaled by W2SCALE
    w2_d = nc.dram_tensor("w2", [2, 2, 128, 2 * 128], fp8,
                          kind="ExternalInput")
    b1_d = nc.dram_tensor("b1", [2, 128, 1], f32, kind="ExternalInput")
    b2_d = nc.dram_tensor("b2", [2, 128, 1], f32, kind="ExternalInput")
    out_d = nc.dram_tensor("out", [128, 2 * mc_per_core], f32,
                           kind="ExternalOutput")

    with tile.TileContext(nc) as tc:
        with tc.tile_pool(name="consts", bufs=1) as cpool, \
             tc.tile_pool(name="work", bufs=4) as pool, \
             tc.tile_pool(name="psum", bufs=2, space="PSUM") as psum:

            w1xa = cpool.tile([DX, 128], bf16, tag="w1xa")
            w1xb = cpool.tile([DX, 128], bf16, tag="w1xb")
            nc.sync.dma_start(w1xa[:], w1x_d[:, 0:128])
            nc.sync.dma_start(w1xb[:], w1x_d[:, 128:256])
            w1za = cpool.tile([DZ, 128], bf16, tag="w1za")
            w1zb = cpool.tile([DZ, 128], bf16, tag="w1zb")
            nc.sync.dma_start(w1za[:], w1z_d[:, 0:128])
            nc.sync.dma_start(w1zb[:], w1z_d[:, 128:256])
            w2t = {}
            for m in range(2):
                for t in range(2):
                    w = cpool.tile([128, 2 * 128], fp8, tag=f"w2_{m}{t}")
                    nc.sync.dma_start(w[:], w2_d[m, t])
                    w2t[m, t] = w[:].rearrange("p (k m) -> p k m", k=2)
            b1a = cpool.tile([128, 1], f32, tag="b1a")
            b1b = cpool.tile([128, 1], f32, tag="b1b")
            b2a = cpool.tile([128, 1], f32, tag="b2a")
            b2b = cpool.tile([128, 1], f32, tag="b2b")
            nc.sync.dma_start(b1a[:], b1_d[0])
            nc.sync.dma_start(b1b[:], b1_d[1])
            nc.sync.dma_start(b2a[:], b2_d[0])
            nc.sync.dma_start(b2b[:], b2_d[1])
            ones = cpool.tile([128, 1], f32, tag="ones")
            nc.vector.memset(ones[:], 1.0)

            out2 = cpool.tile([128, 2 * mc_per_core], f32, tag="out2")

            # two blocks share one DMA + one Ln op (amortize ACT overhead);
            # the Ln is emitted two superblocks ahead so it fills ACT idle
            # time without ever delaying a relu that gates the PE
            def emit_ln(k):
                xt = pool.tile([DX, 2 * BLK], bf16, tag="xt")
                nc.sync.dma_start(xt[:], xt_d[k])
                xl = pool.tile([DX, 2 * BLK], bf16, tag="xl")
                nc.scalar.activation(xl[:], xt[:], Act.Ln, bias=ones[:])
                return xl

            nsb = nblk // 2
            xls_ahead = [emit_ln(0), emit_ln(1) if nsb > 1 else None]
            for sblk in range(nsb):
                xl_cur = xls_ahead.pop(0)
                for half in range(2):
                    blk = 2 * sblk + half
                    xls = xl_cur[:, half * BLK:(half + 1) * BLK]
                    zct = pool.tile([DZ, BLK], bf16, tag="zct")
                    nc.sync.dma_start(zct[:], zct_d[blk])

                    ps1a = psum.tile([128, BLK], f32, tag="ps1a")
                    nc.tensor.matmul(ps1a[:], w1xa[:], xls, start=True, stop=False)
                    nc.tensor.matmul(ps1a[:], w1za[:], zct[:], start=False, stop=True)
                    ps1b = psum.tile([128, BLK], f32, tag="ps1b")
                    nc.tensor.matmul(ps1b[:], w1xb[:], xls, start=True, stop=False)
                    nc.tensor.matmul(ps1b[:], w1zb[:], zct[:], start=False, stop=True)

                    # h1 halves stacked as the two DoubleRow k-tiles, fp8
                    h1 = pool.tile([128, 2 * BLK], fp8, tag="h1")
                    nc.scalar.activation(h1[:, 0:BLK], ps1a[:], Act.Relu,
                                         bias=b1a[:])
                    nc.scalar.activation(h1[:, BLK:2 * BLK], ps1b[:], Act.Relu,
                                         bias=b1b[:])
                    h1v = h1[:].rearrange("p (k c) -> p k c", k=2)

                    # the (2x-scaled) lo-term runs on even blocks only: the
                    # correction is ~3% of scale so 2x-on-half-the-cells is
                    # first-order exact through the relu and the segment mean
                    lo = blk % 2 == 0
                    ps2a = psum.tile([128, BLK], f32, tag="ps2a")
                    nc.tensor.matmul(ps2a[:], w2t[0, 0], h1v, start=True,
                                     stop=not lo, perf_mode=DR)
                    if lo:
                        nc.tensor.matmul(ps2a[:], w2t[0, 1], h1v, start=False,
                                         stop=True, perf_mode=DR)
                    ps2b = psum.tile([128, BLK], f32, tag="ps2b")
                    nc.tensor.matmul(ps2b[:], w2t[1, 0], h1v, start=True,
                                     stop=not lo, perf_mode=DR)
                    if lo:
                        nc.tensor.matmul(ps2b[:], w2t[1, 1], h1v, start=False,
                                         stop=True, perf_mode=DR)

                    h2 = pool.tile([128, 2 * BLK], bf16, tag="h2")
                    nc.vector.tensor_scalar(h2[:, 0:BLK], ps2a[:], b2a[:], 0.0,
                                            op0=Alu.add, op1=Alu.max)
                    nc.vector.tensor_scalar(h2[:, BLK:2 * BLK], ps2b[:], b2b[:],
                                            0.0, op0=Alu.add, op1=Alu.max)

                    # GpSimd pre-folds each 64-cell minichunk in half
                    # (SBUF->SBUF add), halving the DVE reduce read size.
                    h2v = h2[:].rearrange("p (g t m) -> p g t m", t=2, m=MC // 2)
                    h2f = pool.tile([128, BLK], bf16, tag="h2f")
                    h2fv = h2f[:].rearrange("p (g m) -> p g m", m=MC // 2)
                    nc.gpsimd.tensor_tensor(
                        h2fv, h2v[:, :, 0:1, :], h2v[:, :, 1:2, :], op=Alu.add)

                    oslice = slice(blk * 2 * (BLK // MC),
                                   (blk + 1) * 2 * (BLK // MC))
                    nc.vector.tensor_reduce(
                        out2[:, oslice], h2fv,
                        axis=mybir.AxisListType.X, op=Alu.add)
                if sblk + 2 < nsb:
                    xls_ahead.append(emit_ln(sblk + 2))

            nc.sync.dma_start(out_d[:], out2[:])

    nc.compile()
    return nc


def _get_program(nblk):
    if nblk not in _compiled:
        _compiled[nblk] = _build_program(nblk)
    return _compiled[nblk]


def kernel(X, Z, W1, b1, W2, b2, W3, b3, cell_to_batch, sample_idx_batch):
    X = np.asarray(X)
    Z = np.asarray(Z)
    W1 = np.asarray(W1, dtype=np.float32)
    b1 = np.asarray(b1, dtype=np.float32)
    W2 = np.asarray(W2, dtype=np.float32)
    b2 = np.asarray(b2, dtype=np.float32)
    W3 = np.asarray(W3, dtype=np.float32)
    b3 = np.asarray(b3, dtype=np.float32)
    c2b = np.asarray(cell_to_batch).astype(np.int64)
    sib = np.asarray(sample_idx_batch).astype(np.int64)

    n = X.shape[0]
    nseg = sib.shape[0]
    seg = sib[c2b]

    # ---- host layout prep -------------------------------------------------
    order = np.argsort(seg, kind="stable")
    seg_sorted = seg[order]
    counts = np.bincount(seg, minlength=nseg).astype(np.int64)
    padded = ((counts + MC - 1) // MC) * MC
    starts = np.concatenate([[0], np.cumsum(padded)])[:nseg]
    total_pad = int(padded.sum())
    nblk = NBLK
    while total_pad > N_CORES * nblk * BLK:  # safety fallback, recompiles
        nblk += 2
    ntot = N_CORES * nblk * BLK
    mc_per_core = nblk * (BLK // MC)
    run_starts = np.concatenate([[0], np.cumsum(counts)])[:nseg]
    ranks = np.arange(n, dtype=np.int64) - run_starts[seg_sorted]
    slots = starts[seg_sorted] + ranks

    Xs = np.zeros((ntot, DX), dtype=BF16)
    Xs[slots] = X[order].astype(BF16)
    Zs = np.zeros((ntot, DZ), dtype=BF16)
    Zs[slots] = Z[c2b[order]].astype(BF16)

    xt = np.ascontiguousarray(
        Xs.reshape(N_CORES, nblk // 2, 2 * BLK, DX).transpose(0, 1, 3, 2))
    zct = np.ascontiguousarray(
        Zs.reshape(N_CORES, nblk, BLK, DZ).transpose(0, 1, 3, 2))

    n_mc = ntot // MC
    mc_label = np.full(n_mc, -1, dtype=np.int64)
    mc_real = np.zeros(n_mc, dtype=np.int64)
    mc_of_slot = slots // MC
    mc_label[mc_of_slot] = seg_sorted
    np.add.at(mc_real, mc_of_slot, 1)

    # ---- weights ----------------------------------------------------------
    w1x = np.ascontiguousarray(W1[:DX]).astype(BF16)
    w1z = np.ascontiguousarray(W1[DX:DX + DZ]).astype(BF16)
    # W2 as a scaled fp8 (hi, lo) pair; together they are W2 to ~4e-4
    w2f = W2.astype(BF16).astype(np.float32) * W2SCALE
    t_hi = w2f.astype(FP8)
    # lo term ships pre-doubled: it is applied on even blocks only
    t_lo = (2.0 * (w2f - t_hi.astype(np.float32))).astype(FP8)
    w2q = np.zeros((2, 2, 128, 2 * 128), dtype=FP8)
    for m in range(2):
        for t, term in enumerate((t_hi, t_lo)):
            # [p, ktile*128] with element [p, k*128+mc] = term[k*128+p, m*128+mc]
            w2q[m, t] = (term.reshape(2, 128, H).transpose(1, 0, 2)
                         [:, :, m * 128:(m + 1) * 128].reshape(128, 256))
    b1d = np.ascontiguousarray(b1.reshape(2, 128, 1))
    b2d = np.ascontiguousarray(b2.reshape(2, 128, 1)) * W2SCALE

    # ---- run on 8 cores ---------------------------------------------------
    nc = _get_program(nblk)
    in_maps = []
    for c in range(N_CORES):
        in_maps.append({
            "xt": xt[c], "zct": zct[c],
            "w1x": w1x, "w1z": w1z, "w2": w2q, "b1": b1d, "b2": b2d,
        })
    global _last_in_maps
    _last_in_maps = in_maps
    res = run_bass_kernel_spmd(nc, in_maps, list(range(N_CORES)))

    # ---- host epilogue ----------------------------------------------------
    per_core = []
    for c in range(N_CORES):
        o = res.results[c]["out"].reshape(128, nblk, 2, BLK // MC)
        per_core.append(np.concatenate(
            [o[:, :, 0, :].reshape(128, mc_per_core),
             o[:, :, 1, :].reshape(128, mc_per_core)], axis=0))
    sums = np.concatenate(per_core, axis=1)  # [256, n_mc], scaled by W2SCALE

    # analytic contribution of one pad cell (X=0, Z=0), matching device math;
    # even blocks include the doubled lo-term, odd blocks are hi-only
    h1p = np.maximum(b1, 0.0).astype(FP8).astype(np.float32)
    w2eff = t_hi.astype(np.float32) + t_lo.astype(np.float32)
    v_pad_even = np.maximum(h1p @ w2eff + W2SCALE * b2, 0.0) \
        .astype(BF16).astype(np.float32)
    v_pad_odd = np.maximum(h1p @ t_hi.astype(np.float32) + W2SCALE * b2, 0.0) \
        .astype(BF16).astype(np.float32)
    mc_parity = (np.arange(n_mc) // (BLK // MC)) % 2
    v_pad = np.where(mc_parity[None, :] == 0,
                     v_pad_even[:, None], v_pad_odd[:, None])
    sums = sums - v_pad * (MC - mc_real).astype(np.float32)[None, :]
    sums /= W2SCALE

    valid = mc_label >= 0
    S = np.zeros((nseg, H), dtype=np.float32)
    np.add.at(S, mc_label[valid], sums[:, valid].T)

    denom = np.maximum(counts, 1).astype(np.float32)[:, None]
    Y = S @ W3 / denom + b3[None, :]
    Y[counts == 0] = 0.0
    return Y.astype(np.float32)



# revision 3
# speedup vs baseline: 1.4881x; 1.4881x over previous
"""Trainium2 Bass kernel for CompositionModel (gnn_message_passing).

Model: per-cell MLP over [log1p(X) ++ Z[cell_to_batch]] followed by a
segment-mean over batch labels.

v2 strategy (all-fp8 DoubleRow):
  * Host: log1p(X) computed on host, shipped as fp8. Cells sorted by segment,
    padded to 64-cell minichunks; minichunks assigned to even/odd blocks with
    per-segment parity balance (so alternate-block corrections average out
    per segment). Moving tile per 512-cell block is [128, 2, 512] fp8:
      ktile0       = Q8(log1p(X))                    (128 rows)
      ktile1[0:32] = Q8(Zc)            (hi)
      ktile1[32:64]= Q8(16*(Zc-hi))    (lo, exact-ish Z)
      ktile1[64]   = 1.0  -> stationary carries Q8(S1*b1)       (bias hi)
      ktile1[65]   = 1.0  -> stationary carries Q8(residual b1) (bias lo)
      ktile1[66:128] = dup of X rows [0:62] (even blk) / [62:124] (odd blk)
        -> stationary carries 2*(S1*W1x - Q8(S1*W1x)) : the W1 quantization
           error correction applied to half the rows on alternate blocks
           (first-order exact through the segment mean).
  * Device per block: L1 = 2 DR matmuls (K=256 incl. Z+bias+W1lo packed) ->
    ps1 [128,1024] f32 (2 banks) -> single DVE max(x,0) -> fp8 h1 ->
    L2 = 2 DR matmuls hi (+2 lo on even blocks) -> ps2 [128,1024] ->
    2 ACT relu+bias(b2 half) -> bf16 h2 -> GpSimd fold (64->32) ->
    DVE fold (32->16) -> DVE grouped tensor_reduce -> per-minichunk sums.
  * W3/b3 applied on host to the 512x256 segment sums; pad-cell contribution
    subtracted analytically (parity-dependent).
"""

import numpy as np
import ml_dtypes

import concourse.bacc as bacc
import concourse.mybir as mybir
import concourse.tile as tile
from concourse.bass_utils import run_bass_kernel_spmd

BF16 = ml_dtypes.bfloat16
FP8 = ml_dtypes.float8_e4m3fn

N_CORES = 8
DX = 128
DZ = 32
H = 256
B = 512
MC = 64            # minichunk: cells per single-segment group
BLK = 512          # cells per device block
NBLK = 126         # blocks per core
S1 = 32.0          # scale on W1/b1 (fp8 range use)
S2 = 512.0         # scale on W2/b2
NDUP = 62          # X rows corrected per parity

_compiled = {}
_last_in_maps = None


def _q8(a):
    return np.asarray(a, np.float32).astype(FP8)


def _build_program(nblk):
    f32 = mybir.dt.float32
    bf16 = mybir.dt.bfloat16
    fp8 = mybir.dt.float8e4
    Alu = mybir.AluOpType
    Act = mybir.ActivationFunctionType
    DR = mybir.MatmulPerfMode.DoubleRow
    mc_per_core = nblk * (BLK // MC)

    nc = bacc.Bacc("TRN2", target_bir_lowering=False, debug=False,
                   num_devices=N_CORES)

    xz_d = nc.dram_tensor("xz", [nblk, 128, 2 * BLK], fp8, kind="ExternalInput")
    # [parity][mhalf][p, ktile*128] fp8
    w1_d = nc.dram_tensor("w1", [2, 2, 128, 2 * 128], fp8, kind="ExternalInput")
    # [hi/lo][mhalf][p, ktile*128] fp8
    w2_d = nc.dram_tensor("w2", [2, 2, 128, 2 * 128], fp8, kind="ExternalInput")
    b2_d = nc.dram_tensor("b2", [2, 128, 1], f32, kind="ExternalInput")
    out_d = nc.dram_tensor("out", [128, 16 * nblk], f32, kind="ExternalOutput")

    with tile.TileContext(nc) as tc:
        with tc.tile_pool(name="consts", bufs=1) as cpool, \
             tc.tile_pool(name="work", bufs=4) as pool, \
             tc.tile_pool(name="psum", bufs=2, space="PSUM") as psum:

            w1t = {}
            for par in range(2):
                for m in range(2):
                    w = cpool.tile([128, 2 * 128], fp8, tag=f"w1_{par}{m}")
                    nc.sync.dma_start(w[:], w1_d[par, m])
                    w1t[par, m] = w[:].rearrange("p (k m) -> p k m", k=2)
            w2t = {}
            for t in range(2):
                for m in range(2):
                    w = cpool.tile([128, 2 * 128], fp8, tag=f"w2_{t}{m}")
                    nc.sync.dma_start(w[:], w2_d[t, m])
                    w2t[t, m] = w[:].rearrange("p (k m) -> p k m", k=2)
            b2a = cpool.tile([128, 1], f32, tag="b2a")
            b2b = cpool.tile([128, 1], f32, tag="b2b")
            nc.sync.dma_start(b2a[:], b2_d[0])
            nc.sync.dma_start(b2b[:], b2_d[1])

            out2 = cpool.tile([128, 16 * nblk], f32, tag="out2")

            def emit_dma(i):
                xz = pool.tile([128, 2 * BLK], fp8, tag="xz")
                nc.sync.dma_start(xz[:], xz_d[i])
                return xz

            def emit_l1(i, xz):
                ps1 = psum.tile([128, 2 * BLK], f32, tag="ps1")
                xzv = xz[:].rearrange("p (k c) -> p k c", k=2)
                par = i % 2
                nc.tensor.matmul(ps1[:, 0:BLK], w1t[par, 0], xzv,
                                 start=True, stop=True, perf_mode=DR)
                nc.tensor.matmul(ps1[:, BLK:2 * BLK], w1t[par, 1], xzv,
                                 start=True, stop=True, perf_mode=DR)
                return ps1

            def emit_relu1(i, ps1):
                h1 = pool.tile([128, 2 * BLK], fp8, tag="h1")
                nc.vector.tensor_scalar(h1[:], ps1[:], 0.0, None, op0=Alu.max)
                return h1

            def emit_l2(i, h1):
                ps2 = psum.tile([128, 2 * BLK], f32, tag="ps2")
                h1v = h1[:].rearrange("p (k c) -> p k c", k=2)
                lo = i % 2 == 0
                nc.tensor.matmul(ps2[:, 0:BLK], w2t[0, 0], h1v,
                                 start=True, stop=not lo, perf_mode=DR)
                if lo:
                    nc.tensor.matmul(ps2[:, 0:BLK], w2t[1, 0], h1v,
                                     start=False, stop=True, perf_mode=DR)
                nc.tensor.matmul(ps2[:, BLK:2 * BLK], w2t[0, 1], h1v,
                                 start=True, stop=not lo, perf_mode=DR)
                if lo:
                    nc.tensor.matmul(ps2[:, BLK:2 * BLK], w2t[1, 1], h1v,
                                     start=False, stop=True, perf_mode=DR)
                return ps2

            def emit_relu2(i, ps2):
                h2 = pool.tile([128, 2 * BLK], bf16, tag="h2")
                nc.scalar.activation(h2[:, 0:BLK], ps2[:, 0:BLK], Act.Relu,
                                     bias=b2a[:])
                nc.scalar.activation(h2[:, BLK:2 * BLK], ps2[:, BLK:2 * BLK],
                                     Act.Relu, bias=b2b[:])
                return h2

            def emit_fold1(i, h2):
                # 16 groups of 64 -> 32 on GpSimd
                h2v = h2[:].rearrange("p (g t m) -> p g t m", t=2, m=MC // 2)
                h2f = pool.tile([128, BLK], bf16, tag="h2f")
                h2fv = h2f[:].rearrange("p (g m) -> p g m", m=MC // 2)
                nc.gpsimd.tensor_tensor(
                    h2fv, h2v[:, :, 0:1, :], h2v[:, :, 1:2, :], op=Alu.add)
                return h2f

            def emit_fold2(i, h2f):
                # 16 groups of 32 -> 16 on DVE, then grouped reduce to sums
                h2fv = h2f[:].rearrange("p (g t m) -> p g t m", t=2, m=MC // 4)
                h2g = pool.tile([128, BLK // 2], bf16, tag="h2g")
                h2gv = h2g[:].rearrange("p (g m) -> p g m", m=MC // 4)
                nc.vector.tensor_tensor(
                    h2gv, h2fv[:, :, 0:1, :], h2fv[:, :, 1:2, :], op=Alu.add)
                nc.vector.tensor_reduce(
                    out2[:, i * 16:(i + 1) * 16], h2gv,
                    axis=mybir.AxisListType.X, op=Alu.add)

            # ---- software pipeline -------------------------------------
            xzs = {0: emit_dma(0), 1: emit_dma(1)}
            ps1s = {0: emit_l1(0, xzs.pop(0)), 1: emit_l1(1, xzs.pop(1))}
            folds = {}
            for i in range(nblk):
                h1 = emit_relu1(i, ps1s.pop(i))          # DVE
                if i + 2 < nblk:
                    xzs[i + 2] = emit_dma(i + 2)
                    ps1s[i + 2] = emit_l1(i + 2, xzs.pop(i + 2))  # PE
                ps2 = emit_l2(i, h1)                     # PE
                h2 = emit_relu2(i, ps2)                  # ACT x2
                folds[i] = emit_fold1(i, h2)             # GpSimd
                if i - 1 in folds:
                    emit_fold2(i - 1, folds.pop(i - 1))  # DVE x2 (lagged)
            for i in sorted(folds):
                emit_fold2(i, folds.pop(i))

            nc.sync.dma_start(out_d[:], out2[:])

    nc.compile()
    return nc


def _get_program(nblk):
    if nblk not in _compiled:
        _compiled[nblk] = _build_program(nblk)
    return _compiled[nblk]


def kernel(X, Z, W1, b1, W2, b2, W3, b3, cell_to_batch, sample_idx_batch):
    X = np.asarray(X)
    Z = np.asarray(Z)
    W1 = np.asarray(W1, dtype=np.float32)
    b1 = np.asarray(b1, dtype=np.float32)
    W2 = np.asarray(W2, dtype=np.float32)
    b2 = np.asarray(b2, dtype=np.float32)
    W3 = np.asarray(W3, dtype=np.float32)
    b3 = np.asarray(b3, dtype=np.float32)
    c2b = np.asarray(cell_to_batch).astype(np.int64)
    sib = np.asarray(sample_idx_batch).astype(np.int64)

    n = X.shape[0]
    nseg = sib.shape[0]
    seg = sib[c2b]

    # ---- minichunk assignment with per-segment parity balance -------------
    counts = np.bincount(seg, minlength=nseg).astype(np.int64)
    m_seg = (counts + MC - 1) // MC            # minichunks per segment
    M = int(m_seg.sum())
    nblk = NBLK
    while M > N_CORES * nblk * (BLK // MC):    # safety fallback, recompiles
        nblk += 2
    mc_per_core = nblk * (BLK // MC)
    n_mc = N_CORES * mc_per_core
    half_cap = n_mc // 2                        # even-block pool capacity

    # for each segment: alternate its minichunks between even/odd pools
    par_of = np.zeros(M, dtype=np.int64)        # parity of each (seg-ordered) mc
    mc_seg_label = np.zeros(M, dtype=np.int64)
    pos = 0
    tE = tO = 0
    for s in range(nseg):
        m = int(m_seg[s])
        if m == 0:
            continue
        start = 0 if tE <= tO else 1
        pars = (np.arange(m) + start) % 2
        par_of[pos:pos + m] = pars
        mc_seg_label[pos:pos + m] = s
        tE += int(np.sum(pars == 0))
        tO += int(np.sum(pars == 1))
        pos += m
    # index within parity pool, in segment order
    idx_in_pool = np.zeros(M, dtype=np.int64)
    isE = par_of == 0
    idx_in_pool[isE] = np.arange(int(isE.sum()))
    idx_in_pool[~isE] = np.arange(int((~isE).sum()))
    # pool index -> physical mc slot (core, block, mc_in_block)
    per_core_half = mc_per_core // 2            # mc slots of one parity per core
    core = idx_in_pool // per_core_half
    r = idx_in_pool % per_core_half
    block = 2 * (r // (BLK // MC)) + par_of
    mc_in_blk = r % (BLK // MC)
    mc_slot = (core * nblk + block) * (BLK // MC) + mc_in_blk
    assert mc_slot.max() < n_mc and idx_in_pool.max() < half_cap

    mc_label = np.full(n_mc, -1, dtype=np.int64)
    mc_real = np.zeros(n_mc, dtype=np.int64)
    mc_label[mc_slot] = mc_seg_label

    # cells -> slots
    order = np.argsort(seg, kind="stable")
    seg_sorted = seg[order]
    run_starts = np.concatenate([[0], np.cumsum(counts)])[:nseg]
    ranks = np.arange(n, dtype=np.int64) - run_starts[seg_sorted]
    # mc index within segment
    mc_of_cell = ranks // MC
    seg_mc_starts = np.concatenate([[0], np.cumsum(m_seg)])[:nseg]
    mc_id = seg_mc_starts[seg_sorted] + mc_of_cell      # index into M arrays
    slots = mc_slot[mc_id] * MC + (ranks % MC)
    np.add.at(mc_real, mc_slot[mc_id], 1)

    ntot = n_mc * MC

    # ---- data quantization ------------------------------------------------
    Xq = _q8(np.log1p(X, dtype=np.float32))             # [n, 128] fp8
    Zhi = _q8(Z)                                        # [B, 32]
    Zlo = _q8(16.0 * (Z - Zhi.astype(np.float32)))

    Xs = np.zeros((ntot, DX), dtype=FP8)
    Xs[slots] = Xq[order]
    Zs = np.zeros((ntot, 2 * DZ), dtype=FP8)
    Zs[slots, 0:DZ] = Zhi[seg_sorted]
    Zs[slots, DZ:2 * DZ] = Zlo[seg_sorted]

    # per-core/block transposed layout [core, blk, 128, 1024]
    xt = Xs.reshape(N_CORES, nblk, BLK, DX).transpose(0, 1, 3, 2)
    zt = Zs.reshape(N_CORES, nblk, BLK, 2 * DZ).transpose(0, 1, 3, 2)
    xz = np.empty((N_CORES, nblk, 128, 2 * BLK), dtype=FP8)
    xz[:, :, :, 0:BLK] = xt
    xz[:, :, 0:2 * DZ, BLK:2 * BLK] = zt
    xz[:, :, BLK_ONES0, BLK:2 * BLK] = np.float32(1.0)
    xz[:, :, BLK_ONES1, BLK:2 * BLK] = np.float32(1.0)
    # X dup rows (parity-dependent)
    xz[:, 0::2, 2 * DZ + 2:128, BLK:2 * BLK] = xt[:, 0::2, 0:NDUP, :]
    xz[:, 1::2, 2 * DZ + 2:128, BLK:2 * BLK] = xt[:, 1::2, NDUP:2 * NDUP, :]

    # ---- weights ----------------------------------------------------------
    w1s = (S1 * W1).astype(np.float32)                  # [160, 256]
    w1x_hi = _q8(w1s[0:DX])                             # [128, 256]
    w1x_lo = _q8(2.0 * (w1s[0:DX] - w1x_hi.astype(np.float32)))
    w1z_hi = _q8(w1s[DX:DX + DZ])                       # [32, 256]
    w1z_lo = (w1z_hi.astype(np.float32) / 16.0).astype(FP8)
    b1s = (S1 * b1).astype(np.float32)
    b1hi = _q8(b1s)
    b1lo = _q8(b1s - b1hi.astype(np.float32))

    # stationary ktile rows [256, 256] then pack [parity][m][p, k*128+mc]
    w1q = np.zeros((2, 2, 128, 2 * 128), dtype=FP8)
    for par in range(2):
        st = np.zeros((256, H), dtype=FP8)
        st[0:128] = w1x_hi
        st[128 + 0:128 + DZ] = w1z_hi
        st[128 + DZ:128 + 2 * DZ] = w1z_lo
        st[128 + 2 * DZ] = b1hi
        st[128 + 2 * DZ + 1] = b1lo
        st[128 + 2 * DZ + 2:256] = w1x_lo[par * NDUP:(par + 1) * NDUP]
        for m in range(2):
            w1q[par, m] = (st.reshape(2, 128, H).transpose(1, 0, 2)
                           [:, :, m * 128:(m + 1) * 128].reshape(128, 256))

    w2s = (S2 * W2).astype(np.float32)
    t_hi = _q8(w2s)
    t_lo = _q8(2.0 * (w2s - t_hi.astype(np.float32)))
    w2q = np.zeros((2, 2, 128, 2 * 128), dtype=FP8)
    for t, term in enumerate((t_hi, t_lo)):
        for m in range(2):
            w2q[t, m] = (term.reshape(2, 128, H).transpose(1, 0, 2)
                         [:, :, m * 128:(m + 1) * 128].reshape(128, 256))
    b2d = np.ascontiguousarray((S1 * S2 * b2).reshape(2, 128, 1)) \
        .astype(np.float32)

    # ---- run on 8 cores ---------------------------------------------------
    nc = _get_program(nblk)
    in_maps = []
    for c in range(N_CORES):
        in_maps.append({"xz": xz[c], "w1": w1q, "w2": w2q, "b2": b2d})
    global _last_in_maps
    _last_in_maps = in_maps
    res = run_bass_kernel_spmd(nc, in_maps, list(range(N_CORES)))

    # ---- host epilogue ----------------------------------------------------
    per_core = []
    for c in range(N_CORES):
        o = res.results[c]["out"].reshape(128, nblk, 2, BLK // MC)
        per_core.append(np.concatenate(
            [o[:, :, 0, :].reshape(128, mc_per_core),
             o[:, :, 1, :].reshape(128, mc_per_core)], axis=0))
    sums = np.concatenate(per_core, axis=1)   # [256, n_mc], scaled S1*S2

    # analytic pad-cell contribution (X=0, Z=0, ones=1), parity-dependent
    pre1_pad = b1hi.astype(np.float32) + b1lo.astype(np.float32)  # [256]
    h1_pad = _q8(np.maximum(pre1_pad, 0.0)).astype(np.float32)
    w2hi_f = t_hi.astype(np.float32)
    w2lo_f = t_lo.astype(np.float32)
    b2s = (S1 * S2 * b2).astype(np.float32)
    v_pad_even = np.maximum(h1_pad @ (w2hi_f + w2lo_f) + b2s, 0.0) \
        .astype(BF16).astype(np.float32)
    v_pad_odd = np.maximum(h1_pad @ w2hi_f + b2s, 0.0) \
        .astype(BF16).astype(np.float32)
    blk_of_mc = (np.arange(n_mc) // (BLK // MC)) % nblk
    mc_parity = blk_of_mc % 2
    v_pad = np.where(mc_parity[None, :] == 0,
                     v_pad_even[:, None], v_pad_odd[:, None])
    sums = sums - v_pad * (MC - mc_real).astype(np.float32)[None, :]
    sums /= np.float32(S1 * S2)

    valid = mc_label >= 0
    S = np.zeros((nseg, H), dtype=np.float32)
    np.add.at(S, mc_label[valid], sums[:, valid].T)

    denom = np.maximum(counts, 1).astype(np.float32)[:, None]
    Y = S @ W3 / denom + b3[None, :]
    Y[counts == 0] = 0.0
    return Y.astype(np.float32)


BLK_ONES0 = 2 * DZ      # row 64 of ktile1 (ones -> b1 hi)
BLK_ONES1 = 2 * DZ + 1  # row 65 (ones -> b1 lo)
